# revision 25
# baseline (speedup 1.0000x reference)
"""Trainium2 Bass kernel: 3x3 SAME conv (64->128ch) + bias, double-tanh, min over
channels, for x[16,64,224,224] -> y[16,1,224,224].

Strategy
--------
- Data-parallel over batch: 16 images / 8 NeuronCores = 2 images per core.
  Same NEFF on every core, different input shard (no collectives).
- min_c tanh(tanh(v_c)) == tanh(tanh(min_c v_c)) (tanh is monotone), so the
  double tanh is applied only to the per-pixel channel-minimum.
- Conv as implicit GEMM with the *image patch stationary*: tiles of M=128
  consecutive pixels of the padded row-stream (row stride 226; the 2 pad cols
  per row produce garbage outputs that are dropped at extraction), accumulated
  into PSUM[128, 128oc] with matmuls lhsT=[K, 128 px], rhs=[K, 128 oc].
  M=128 keeps NumWeights==128 so the compiler enables Fast Weight Load.
  Output channels land on the PSUM free dim, so the channel-min is a native
  free-dim DVE reduction.
- A single strip tile per 28 output rows carries all nine taps: partitions
  0:64 = padded rows r..r+30, partitions 64:128 = rows r+1..r+31 (the same
  channels one row later).  Per pixel tile:
    * (kh=0,kw)|(kh=1,kw) K=128 pairs at window kw, kw = 0,1,2  (3 matmuls)
    * the three (kh=2,kw) taps are K=64 singles: for tile q from the upper
      half at window 2*226+kw, for tile q+4 from the lower half at window
      226+kw.  They are emitted alternating upper/lower so consecutive
      matmuls occupy disjoint PE row groups (tile_position row packing) and
      run concurrently into different PSUM banks -> ~1.5 matmuls/tile.
  Total PE stream ~4.5 x 128 columns per 128-pixel tile = the 9-tap minimum.
- Bias: each 8-tile / 2-bank PSUM group is pre-seeded with the bias, so conv
  matmuls accumulate onto it (start=False) and the DVE only does reduce_min
  over the oc axis.  The seed is a ScalarE copy into PSUM — legal because the
  PSUM has_written bits stay set from the slot's previous accumulation group;
  the first CPSUM_BUFS groups are instead seeded by K=1 rank-1 matmuls
  (ones stationary, bias streaming, start=True) to initialize the bits.
- Minima collect in a stage tile that is PE-transposed so pixels become the
  free dim, double-tanh'd on ScalarE, and DMA'd to a DRAM scratch in
  padded-stream order; one strided DRAM->DRAM DMA per image extracts the
  valid 224x224.
- Host-side prep (cheap numpy): zero-pad x to [.,64,228,226] bf16,
  pre-transpose the weights into [128,128] rhs tiles, tile the bias to
  [128,8,128] f32.
"""

import numpy as np
import ml_dtypes

import concourse.bass as bass
import concourse.mybir as mybir
import concourse.tile as tile
from concourse import bacc
from concourse.bass_utils import run_bass_kernel_spmd
from concourse.masks import make_identity

N_CORES = 8
B = 16
BPC = B // N_CORES  # images per core
IC, OC = 64, 128
H = W = 224
PW = 226    # padded row width in the pixel stream (= xp/xs width)
PH = 228    # xp rows (row r = image row r-1; rows 0, 225, 226, 227 zero)
R = 28      # output rows per strip
M = 128     # pixels per matmul tile
NPOS = R + 3   # padded-row positions in the main strip (upper half)
GTOT = H * PW  # padded-stream length per image (50624)
NT = -(-GTOT // M)  # tiles per image (396)
CH = 128    # stage chunk size (tiles per transpose)
F32 = mybir.dt.float32

DT = mybir.dt.bfloat16
DT_NP = ml_dtypes.bfloat16
# Image-data dtype.  fp8-e4m3 was tried (numerically fine — the conv output
# feeds tanh(tanh(min)) which saturates for randn inputs) but measured ~20%
# SLOWER than bf16: the 1-byte-sliding stationary windows appear to break
# Fast Weight Load, leaving the PE LDWEIGHTS-bound.  Keep bf16.
DT_X = mybir.dt.bfloat16
DT_X_NP = ml_dtypes.bfloat16

# Engine-attribution probes (invalid outputs; timing only).
PROBE_DVE_LIGHT = False  # reduce only 8 oc per psum slot (DVE ~5x lighter)
PROBE_DMA_LIGHT = False  # load strips only for s==0, reuse for all strips

# Store x strips as fp8 in HBM and cast to bf16 during the DMA (SWDGE cast
# path).  Halves HBM-side strip traffic; PE still sees bf16 operands.
DMA_CAST_FP8 = False
SINGLE_DMA_STRIP = False

STRIP_BUFS = 6
LOOKAHEAD = 4  # strips prefetched ahead of use (< STRIP_BUFS - 1)
# Seed the bias into each PSUM group with a ScalarE (ACT) copy instead of
# rank-1 matmuls.  PE accumulate (start=False) adds on top because the PSUM
# has_written bits persist from the slot's previous accumulation group; the
# first 3 groups (one per cpsum pool slot) still use the rank-1 matmul with
# start=True to put the bits into a known-set state.
ACT_BIAS = True
CPSUM_BUFS = 3  # PSUM group pool depth (ACT_BIAS seeds the first CPSUM_BUFS
                # groups via rank-1 matmuls to initialize each slot's bits)

_CACHE: dict = {}
LAST_RESULT = None  # BassKernelResults of the most recent run (for profiling)


def _strip_of(t):
    """Strip index owning tile t (by its first pixel row)."""
    return min((t * M) // PW // R, H // R - 1)


def _emit(nc: bass.Bass, tc: tile.TileContext, y, xp, wv, ws, bm,
          n_img=BPC, n_strips=None, nrep=1):
    """Emit the per-core program.

    y  : [n_img, 1, 224, 224] f32   ExternalOutput
    xp : [n_img, 2, 64, 228, 226] DT  padded input; [:,1] = shifted 1 row up
    wv : [3, 128, 128] DT   rhs tiles for the (kh=0 | kh=1) K-pairs, kw=0..2
    ws : [3, 128, 128] DT   ws[kw]: w(2,kw).T replicated in rows 0:64 and
                            64:128 (upper- and lower-half kh=2 singles)
    bm : [128, 8, 128] f32  bias broadcast to partitions and 8 bank slots
    """
    if n_strips is None:
        n_strips = H // R
    n_tiles = NT if n_strips == H // R else ((n_strips * R * PW) // M)
    with (
        tc.tile_pool(name="consts", bufs=1) as cpool,
        tc.tile_pool(name="strips", bufs=STRIP_BUFS) as spool,
        tc.tile_pool(name="stage", bufs=4) as stpool,
        tc.tile_pool(name="obuf", bufs=4) as opool,
        tc.tile_pool(name="dscratch", bufs=2, space="DRAM") as dpool,
        tc.tile_pool(name="cpsum", bufs=CPSUM_BUFS, space="PSUM") as cpsum,
        tc.tile_pool(name="tpsum", bufs=2, space="PSUM") as tpsum,
    ):
        # ---- constants ----
        identity = cpool.tile([128, 128], F32)
        make_identity(nc, identity)
        wv_sb = cpool.tile([128, 3, 128], DT)
        nc.sync.dma_start(wv_sb[:], wv.rearrange("t k n -> k t n"))
        ws_sb = cpool.tile([128, 3, 128], DT)
        nc.sync.dma_start(ws_sb[:], ws.rearrange("t k n -> k t n"))
        bias_mat = cpool.tile([128, 8, 128], DT)
        nc.sync.dma_start(bias_mat[:], bm)
        ones1 = cpool.tile([1, 128], DT)
        nc.gpsimd.memset(ones1[:], 1.0)
        bias_rhs = cpool.tile([1, 4, 128], DT)
        nc.vector.tensor_copy(bias_rhs[:], bias_mat[0:1, 0:4, :])

        grp_idx = 0
        img_list = [bb for _ in range(nrep) for bb in range(n_img)]
        dma = (nc.gpsimd.dma_start if DMA_CAST_FP8 else nc.sync.dma_start)

        # Strip prefetch: one tile per strip carries all nine taps (upper =
        # padded rows h0..h0+30, lower = rows h0+1..h0+31).  Strips are
        # loaded LOOKAHEAD ahead of use, across image boundaries, so the PE
        # never waits on a strip DMA.
        strip_seq = [(bb, sL) for bb in img_list for sL in range(n_strips)]
        strip_aps = [None] * len(strip_seq)
        next_load = [0]

        def load_ahead(upto):
            while next_load[0] <= min(upto, len(strip_seq) - 1):
                i = next_load[0]
                bb, sL = strip_seq[i]
                if PROBE_DMA_LIGHT and sL > 0:
                    strip_aps[i] = strip_aps[i - sL]
                else:
                    h0 = sL * R
                    ss = spool.tile([128, NPOS, PW], DT_X, name="ss")
                    if SINGLE_DMA_STRIP:
                        # ONE 128-partition dma_start per strip (reaches all
                        # 16 SBUF AXI ports; two 64-partition transfers would
                        # serialize on the FIFO rings).  xp carries the
                        # row-shifted copy at h=1 so both halves are one
                        # source.
                        dma(ss[:],
                            xp[bb].rearrange("h c r w -> (h c) r w")
                            [:, h0:h0 + NPOS, 0:PW])
                    else:
                        dma(ss[0:64], xp[bb, 0, :, h0:h0 + NPOS, 0:PW])
                        dma(ss[64:128],
                            xp[bb, 0, :, h0 + 1:h0 + NPOS + 1, 0:PW])
                    strip_aps[i] = ss.rearrange("p a c -> p (a c)")
                next_load[0] += 1

        for bi, b in enumerate(img_list):
            ypad = dpool.tile([NT * M], F32, name="ypad")
            stage_t = None
            psum_t = None
            ssf = None
            cur_strip = -1
            for t in range(n_tiles):
                s = _strip_of(t)
                if s != cur_strip:
                    cur_strip = s
                    si = bi * n_strips + s
                    load_ahead(si + LOOKAHEAD)
                    ssf = strip_aps[si]

                g = t * M - (s * R) * PW  # strip-local stream offset
                q = t % 8
                if q == 0:
                    psum_t = cpsum.tile([M, 8, 128], F32, name="psum_t")
                    if True:
                        pt_flat = psum_t.rearrange("p q n -> p (q n)")
                        if ACT_BIAS and grp_idx >= CPSUM_BUFS:
                            # ScalarE re-seed: values overwritten, has_written
                            # bits stay set from this slot's previous group,
                            # so start=False matmuls accumulate on top.
                            nc.scalar.activation(
                                pt_flat[:],
                                bias_mat.rearrange("p q n -> p (q n)")[:],
                                mybir.ActivationFunctionType.Copy)
                        else:
                            # seed both banks with bias via rank-1 matmuls
                            ngrp = min(8, n_tiles - t)
                            for bk in range(0, ngrp, 4):
                                w4 = min(4, ngrp - bk) * 128
                                nc.tensor.matmul(
                                    pt_flat[:, bk * 128: bk * 128 + w4],
                                    ones1[:],
                                    bias_rhs.rearrange("p q n -> p (q n)")
                                    [:, 0:w4],
                                    start=True, stop=False)
                    grp_idx += 1
                if q == 0:
                    pend = []  # deferred kh=2 singles: (q, g, ssf)
                for kw in range(3):      # (kh=0 | kh=1) pairs
                    nc.tensor.matmul(
                        psum_t[:, q], ssf[:, g + kw: g + kw + M], wv_sb[:, kw],
                        start=False, stop=False)
                # all three (kh=2, kw) taps ride in deferred K=64 singles:
                # tile q from the upper strip half (window 2*PW+kw), tile q+4
                # from the lower half (window PW+kw); emitted alternating so
                # disjoint PE row groups overlap, into different PSUM banks.
                pend.append((q, g, ssf))
                if q == 7 or t == n_tiles - 1:
                    ngrp = q + 1
                    for i in range(4):
                        for kw in range(3):
                            if i < ngrp:
                                qa, ga, fa = pend[i]
                                nc.tensor.matmul(
                                    psum_t[:, qa],
                                    fa[0:64, ga + 2 * PW + kw:
                                       ga + 2 * PW + kw + M],
                                    ws_sb[0:64, kw], start=False,
                                    stop=(kw == 2 and
                                          i == min(3, ngrp - 1)))
                            if i + 4 < ngrp:
                                qb, gb, fb = pend[i + 4]
                                nc.tensor.matmul(
                                    psum_t[:, qb],
                                    fb[64:128, gb + PW + kw:
                                       gb + PW + kw + M],
                                    ws_sb[64:128, kw], start=False,
                                    stop=(kw == 2 and i + 4 == ngrp - 1))

                if t % CH == 0:
                    stage_t = stpool.tile([128, CH], F32, name="stage_t")
                if q == 7 or t == n_tiles - 1:
                    nq = q + 1
                    cc = (t - q) % CH
                    # probe: span every psum slot (keeps the dependency on all
                    # of the group's matmuls — reading a partial slot set
                    # races DVE-R against PE-W in the same bank, which is a
                    # fatal PSUM collision) but read only 8 oc per slot.
                    pin = psum_t[:, 0:nq, 0:8] if PROBE_DVE_LIGHT \
                        else psum_t[:, 0:nq]
                    nc.vector.tensor_reduce(
                        out=stage_t[:, cc:cc + nq],
                        in_=pin,
                        axis=mybir.AxisListType.X,
                        op=mybir.AluOpType.min)
                if t % CH == CH - 1 or t == n_tiles - 1:
                    # chunk done: transpose -> tanh -> tanh -> scratch DMA
                    j = t // CH
                    w = t % CH + 1  # columns written in this chunk
                    tp = tpsum.tile([CH, 128], F32, name="tp")
                    nc.tensor.transpose(tp[0:w, :], stage_t[:, 0:w], identity)
                    ob = opool.tile([CH, 128], F32, name="ob")
                    nc.scalar.activation(
                        ob[0:w, :], tp[0:w, :],
                        mybir.ActivationFunctionType.Tanh)
                    nc.scalar.activation(
                        ob[0:w, :], ob[0:w, :],
                        mybir.ActivationFunctionType.Tanh)
                    nc.sync.dma_start(
                        ypad.rearrange("(t p) -> t p", p=M)[j * CH:j * CH + w],
                        ob[0:w, :])
            # extract valid pixels: drop the 2 pad cols per padded row
            rows_out = (n_tiles * M) // PW  # complete rows (224 when full)
            nc.sync.dma_start(
                y[b, 0, 0:rows_out, :],
                ypad[0:GTOT].rearrange("(h c) -> h c", c=PW)[0:rows_out, 0:W])


def _build(n_img=BPC, n_strips=None, enable_asserts=False, nrep=1):
    # num_devices=1: pure data-parallel SPMD, no collectives — each core runs
    # an independent single-device NEFF on its own input shard.
    nc = bacc.Bacc(
        "TRN2",
        target_bir_lowering=False,
        debug=False,
        enable_asserts=enable_asserts,
        num_devices=1,
    )
    DT_HBM = mybir.dt.float8e4 if DMA_CAST_FP8 else DT_X
    xp = nc.dram_tensor("xp", [n_img, 2, IC, PH, PW], DT_HBM,
                        kind="ExternalInput")
    wv = nc.dram_tensor("wv", [3, 128, 128], DT, kind="ExternalInput")
    ws = nc.dram_tensor("ws", [3, 128, 128], DT, kind="ExternalInput")
    bm = nc.dram_tensor("bias_mat", [128, 8, 128], DT,
                        kind="ExternalInput")
    y = nc.dram_tensor("y", [n_img, 1, H, W], F32, kind="ExternalOutput")
    with tile.TileContext(nc) as tc:
        _emit(nc, tc, y.ap(), xp.ap(), wv.ap(), ws.ap(), bm.ap(),
              n_img=n_img, n_strips=n_strips, nrep=nrep)
    nc.compile()
    return nc


def prep_inputs(x, weight, bias):
    """Host-side layout prep (numpy only)."""
    x = np.asarray(x, dtype=np.float32)
    weight = np.asarray(weight, dtype=np.float32)
    bias = np.asarray(bias, dtype=np.float32)
    nb = x.shape[0]
    xpad = np.zeros((nb, IC, PH, PW), dtype=np.float32)
    xpad[:, :, 1:225, 1:225] = x
    xpad = xpad.astype(
        ml_dtypes.float8_e4m3 if DMA_CAST_FP8 else DT_X_NP)
    # xp[:, 0] = xpad, xp[:, 1] = xpad shifted up one row (the strip tile's
    # lower half), so one 128-partition DMA covers both halves of a strip.
    xp = np.zeros((nb, 2, IC, PH, PW), dtype=xpad.dtype)
    xp[:, 0] = xpad
    xp[:, 1, :, 0:PH - 1] = xpad[:, :, 1:PH]
    xp = np.ascontiguousarray(xp)
    wv = np.zeros((3, 128, 128), dtype=np.float32)
    ws = np.zeros((3, 128, 128), dtype=np.float32)
    for kw in range(3):
        wv[kw, 0:64] = weight[:, :, 0, kw].T
        wv[kw, 64:128] = weight[:, :, 1, kw].T
        ws[kw, 0:64] = weight[:, :, 2, kw].T
        ws[kw, 64:128] = weight[:, :, 2, kw].T
    wv = np.ascontiguousarray(wv.astype(DT_NP))
    ws = np.ascontiguousarray(ws.astype(DT_NP))
    bm = np.ascontiguousarray(
        np.broadcast_to(bias[None, None, :], (128, 8, 128)).astype(DT_NP))
    return xp, wv, ws, bm


def kernel(x, weight, bias):
    global LAST_RESULT
    xp, wv, ws, bm = prep_inputs(x, weight, bias)
    if "nc" not in _CACHE:
        _CACHE["nc"] = _build()
    nc = _CACHE["nc"]
    in_maps = []
    for c in range(N_CORES):
        m = {
            "xp": np.ascontiguousarray(xp[c * BPC:(c + 1) * BPC]),
            "wv": wv,
            "ws": ws,
            "bias_mat": bm,
        }
        in_maps.append(m)
    res = run_bass_kernel_spmd(nc, in_maps, core_ids=list(range(N_CORES)))
    LAST_RESULT = res
    y = np.concatenate([r["y"] for r in res.results], axis=0)
    return y



# revision 26
# speedup vs baseline: 1.0177x; 1.0177x over previous
"""Trainium2 Bass kernel: 3x3 SAME conv (64->128ch) + bias, double-tanh, min over
channels, for x[16,64,224,224] -> y[16,1,224,224].

Strategy
--------
- Data-parallel over batch: 16 images / 8 NeuronCores = 2 images per core.
  Same NEFF on every core, different input shard (no collectives).
- min_c tanh(tanh(v_c)) == tanh(tanh(min_c v_c)) (tanh is monotone), so the
  double tanh is applied only to the per-pixel channel-minimum.
- Conv as implicit GEMM with the *image patch stationary*: tiles of M=128
  consecutive pixels of the padded row-stream (row stride 226; the 2 pad cols
  per row produce garbage outputs that are dropped at extraction), accumulated
  into PSUM[128, 128oc] with matmuls lhsT=[K, 128 px], rhs=[K, 128 oc].
  M=128 keeps NumWeights==128 so the compiler enables Fast Weight Load.
  Output channels land on the PSUM free dim, so the channel-min is a native
  free-dim DVE reduction.
- A single strip tile per 28 output rows carries all nine taps: partitions
  0:64 = padded rows r..r+30, partitions 64:128 = rows r+1..r+31 (the same
  channels one row later).  Per pixel tile:
    * (kh=0,kw)|(kh=1,kw) K=128 pairs at window kw, kw = 0,1,2  (3 matmuls)
    * the three (kh=2,kw) taps are K=64 singles: for tile q from the upper
      half at window 2*226+kw, for tile q+4 from the lower half at window
      226+kw.  They are emitted alternating upper/lower so consecutive
      matmuls occupy disjoint PE row groups (tile_position row packing) and
      run concurrently into different PSUM banks -> ~1.5 matmuls/tile.
  Total PE stream ~4.5 x 128 columns per 128-pixel tile = the 9-tap minimum.
- Bias: each 8-tile / 2-bank PSUM group is pre-seeded with the bias, so conv
  matmuls accumulate onto it (start=False) and the DVE only does reduce_min
  over the oc axis.  The seed is a ScalarE copy into PSUM — legal because the
  PSUM has_written bits stay set from the slot's previous accumulation group;
  the first CPSUM_BUFS groups are instead seeded by K=1 rank-1 matmuls
  (ones stationary, bias streaming, start=True) to initialize the bits.
- Minima collect in a stage tile that is PE-transposed so pixels become the
  free dim, double-tanh'd on ScalarE, and DMA'd to a DRAM scratch in
  padded-stream order; one strided DRAM->DRAM DMA per image extracts the
  valid 224x224.
- Host-side prep (cheap numpy): zero-pad x to [.,64,228,226] bf16,
  pre-transpose the weights into [128,128] rhs tiles, tile the bias to
  [128,8,128] f32.
"""

import numpy as np
import ml_dtypes

import concourse.bass as bass
import concourse.mybir as mybir
import concourse.tile as tile
from concourse import bacc
from concourse.bass_utils import run_bass_kernel_spmd
from concourse.masks import make_identity

N_CORES = 8
B = 16
BPC = B // N_CORES  # images per core
IC, OC = 64, 128
H = W = 224
PW = 226    # padded row width in the pixel stream (= xp/xs width)
PH = 228    # xp rows (row r = image row r-1; rows 0, 225, 226, 227 zero)
R = 28      # output rows per strip
M = 128     # pixels per matmul tile
NPOS = R + 3   # padded-row positions in the main strip (upper half)
GTOT = H * PW  # padded-stream length per image (50624)
NT = -(-GTOT // M)  # tiles per image (396)
CH = 128    # stage chunk size (tiles per transpose)
F32 = mybir.dt.float32

DT = mybir.dt.bfloat16
DT_NP = ml_dtypes.bfloat16
# Image-data dtype.  fp8-e4m3 was tried (numerically fine — the conv output
# feeds tanh(tanh(min)) which saturates for randn inputs) but measured ~20%
# SLOWER than bf16: the 1-byte-sliding stationary windows appear to break
# Fast Weight Load, leaving the PE LDWEIGHTS-bound.  Keep bf16.
DT_X = mybir.dt.bfloat16
DT_X_NP = ml_dtypes.bfloat16

# Engine-attribution probes (invalid outputs; timing only).
PROBE_DVE_LIGHT = False  # reduce only 8 oc per psum slot (DVE ~5x lighter)
PROBE_DMA_LIGHT = False  # load strips only for s==0, reuse for all strips

# Store x strips as fp8 in HBM and cast to bf16 during the DMA (SWDGE cast
# path).  Halves HBM-side strip traffic; PE still sees bf16 operands.
DMA_CAST_FP8 = False
SINGLE_DMA_STRIP = True

STRIP_BUFS = 4
LOOKAHEAD = 2  # strips prefetched ahead of use (< STRIP_BUFS - 1)
# Seed the bias into each PSUM group with a ScalarE (ACT) copy instead of
# rank-1 matmuls.  PE accumulate (start=False) adds on top because the PSUM
# has_written bits persist from the slot's previous accumulation group; the
# first 3 groups (one per cpsum pool slot) still use the rank-1 matmul with
# start=True to put the bits into a known-set state.
ACT_BIAS = True
CPSUM_BUFS = 3  # PSUM group pool depth (ACT_BIAS seeds the first CPSUM_BUFS
                # groups via rank-1 matmuls to initialize each slot's bits)

_CACHE: dict = {}
LAST_RESULT = None  # BassKernelResults of the most recent run (for profiling)


def _strip_of(t):
    """Strip index owning tile t (by its first pixel row)."""
    return min((t * M) // PW // R, H // R - 1)


def _emit(nc: bass.Bass, tc: tile.TileContext, y, xp, wv, ws, bm,
          n_img=BPC, n_strips=None, nrep=1):
    """Emit the per-core program.

    y  : [n_img, 1, 224, 224] f32   ExternalOutput
    xp : [n_img, 2, 64, 228, 226] DT  padded input; [:,1] = shifted 1 row up
    wv : [3, 128, 128] DT   rhs tiles for the (kh=0 | kh=1) K-pairs, kw=0..2
    ws : [3, 128, 128] DT   ws[kw]: w(2,kw).T replicated in rows 0:64 and
                            64:128 (upper- and lower-half kh=2 singles)
    bm : [128, 8, 128] f32  bias broadcast to partitions and 8 bank slots
    """
    if n_strips is None:
        n_strips = H // R
    n_tiles = NT if n_strips == H // R else ((n_strips * R * PW) // M)
    with (
        tc.tile_pool(name="consts", bufs=1) as cpool,
        tc.tile_pool(name="strips", bufs=STRIP_BUFS) as spool,
        tc.tile_pool(name="stage", bufs=4) as stpool,
        tc.tile_pool(name="obuf", bufs=4) as opool,
        tc.tile_pool(name="dscratch", bufs=2, space="DRAM") as dpool,
        tc.tile_pool(name="cpsum", bufs=CPSUM_BUFS, space="PSUM") as cpsum,
        tc.tile_pool(name="tpsum", bufs=2, space="PSUM") as tpsum,
    ):
        # ---- constants ----
        identity = cpool.tile([128, 128], F32)
        make_identity(nc, identity)
        wv_sb = cpool.tile([128, 3, 128], DT)
        nc.sync.dma_start(wv_sb[:], wv.rearrange("t k n -> k t n"))
        ws_sb = cpool.tile([128, 3, 128], DT)
        nc.sync.dma_start(ws_sb[:], ws.rearrange("t k n -> k t n"))
        bias_mat = cpool.tile([128, 8, 128], F32)
        nc.sync.dma_start(bias_mat[:], bm)
        ones1 = cpool.tile([1, 128], DT)
        nc.gpsimd.memset(ones1[:], 1.0)
        bias_rhs = cpool.tile([1, 4, 128], DT)
        nc.vector.tensor_copy(bias_rhs[:], bias_mat[0:1, 0:4, :])

        grp_idx = 0
        img_list = [bb for _ in range(nrep) for bb in range(n_img)]
        dma = (nc.gpsimd.dma_start if DMA_CAST_FP8 else nc.sync.dma_start)

        # Strip prefetch: one tile per strip carries all nine taps (upper =
        # padded rows h0..h0+30, lower = rows h0+1..h0+31).  Strips are
        # loaded LOOKAHEAD ahead of use, across image boundaries, so the PE
        # never waits on a strip DMA.
        strip_seq = [(bb, sL) for bb in img_list for sL in range(n_strips)]
        strip_aps = [None] * len(strip_seq)
        next_load = [0]

        def load_ahead(upto):
            while next_load[0] <= min(upto, len(strip_seq) - 1):
                i = next_load[0]
                bb, sL = strip_seq[i]
                if PROBE_DMA_LIGHT and sL > 0:
                    strip_aps[i] = strip_aps[i - sL]
                else:
                    h0 = sL * R
                    ss = spool.tile([128, NPOS, PW], DT_X, name="ss")
                    if SINGLE_DMA_STRIP:
                        # ONE 128-partition dma_start per strip (reaches all
                        # 16 SBUF AXI ports; two 64-partition transfers would
                        # serialize on the FIFO rings).  xp carries the
                        # row-shifted copy at h=1 so both halves are one
                        # source.
                        dma(ss[:],
                            xp[bb].rearrange("h c r w -> (h c) r w")
                            [:, h0:h0 + NPOS, 0:PW])
                    else:
                        dma(ss[0:64], xp[bb, 0, :, h0:h0 + NPOS, 0:PW])
                        dma(ss[64:128],
                            xp[bb, 0, :, h0 + 1:h0 + NPOS + 1, 0:PW])
                    strip_aps[i] = ss.rearrange("p a c -> p (a c)")
                next_load[0] += 1

        for bi, b in enumerate(img_list):
            ypad = dpool.tile([NT * M], F32, name="ypad")
            stage_t = None
            psum_t = None
            ssf = None
            cur_strip = -1
            for t in range(n_tiles):
                s = _strip_of(t)
                if s != cur_strip:
                    cur_strip = s
                    si = bi * n_strips + s
                    load_ahead(si + LOOKAHEAD)
                    ssf = strip_aps[si]

                g = t * M - (s * R) * PW  # strip-local stream offset
                q = t % 8
                if q == 0:
                    psum_t = cpsum.tile([M, 8, 128], F32, name="psum_t")
                    if True:
                        pt_flat = psum_t.rearrange("p q n -> p (q n)")
                        if ACT_BIAS and grp_idx >= CPSUM_BUFS:
                            # ScalarE re-seed: values overwritten, has_written
                            # bits stay set from this slot's previous group,
                            # so start=False matmuls accumulate on top.
                            nc.scalar.activation(
                                pt_flat[:],
                                bias_mat.rearrange("p q n -> p (q n)")[:],
                                mybir.ActivationFunctionType.Copy)
                        else:
                            # seed both banks with bias via rank-1 matmuls
                            ngrp = min(8, n_tiles - t)
                            for bk in range(0, ngrp, 4):
                                w4 = min(4, ngrp - bk) * 128
                                nc.tensor.matmul(
                                    pt_flat[:, bk * 128: bk * 128 + w4],
                                    ones1[:],
                                    bias_rhs.rearrange("p q n -> p (q n)")
                                    [:, 0:w4],
                                    start=True, stop=False)
                    grp_idx += 1
                if q == 0:
                    pend = []  # deferred kh=2 singles: (q, g, ssf)
                for kw in range(3):      # (kh=0 | kh=1) pairs
                    nc.tensor.matmul(
                        psum_t[:, q], ssf[:, g + kw: g + kw + M], wv_sb[:, kw],
                        start=False, stop=False)
                # all three (kh=2, kw) taps ride in deferred K=64 singles:
                # tile q from the upper strip half (window 2*PW+kw), tile q+4
                # from the lower half (window PW+kw); emitted alternating so
                # disjoint PE row groups overlap, into different PSUM banks.
                pend.append((q, g, ssf))
                if q == 7 or t == n_tiles - 1:
                    ngrp = q + 1
                    for i in range(4):
                        for kw in range(3):
                            if i < ngrp:
                                qa, ga, fa = pend[i]
                                nc.tensor.matmul(
                                    psum_t[:, qa],
                                    fa[0:64, ga + 2 * PW + kw:
                                       ga + 2 * PW + kw + M],
                                    ws_sb[0:64, kw], start=False,
                                    stop=(kw == 2 and
                                          i == min(3, ngrp - 1)))
                            if i + 4 < ngrp:
                                qb, gb, fb = pend[i + 4]
                                nc.tensor.matmul(
                                    psum_t[:, qb],
                                    fb[64:128, gb + PW + kw:
                                       gb + PW + kw + M],
                                    ws_sb[64:128, kw], start=False,
                                    stop=(kw == 2 and i + 4 == ngrp - 1))

                if t % CH == 0:
                    stage_t = stpool.tile([128, CH], F32, name="stage_t")
                if q == 7 or t == n_tiles - 1:
                    nq = q + 1
                    cc = (t - q) % CH
                    # probe: span every psum slot (keeps the dependency on all
                    # of the group's matmuls — reading a partial slot set
                    # races DVE-R against PE-W in the same bank, which is a
                    # fatal PSUM collision) but read only 8 oc per slot.
                    pin = psum_t[:, 0:nq, 0:8] if PROBE_DVE_LIGHT \
                        else psum_t[:, 0:nq]
                    nc.vector.tensor_reduce(
                        out=stage_t[:, cc:cc + nq],
                        in_=pin,
                        axis=mybir.AxisListType.X,
                        op=mybir.AluOpType.min)
                if t % CH == CH - 1 or t == n_tiles - 1:
                    # chunk done: transpose -> tanh -> tanh -> scratch DMA
                    j = t // CH
                    w = t % CH + 1  # columns written in this chunk
                    tp = tpsum.tile([CH, 128], F32, name="tp")
                    nc.tensor.transpose(tp[0:w, :], stage_t[:, 0:w], identity)
                    ob = opool.tile([CH, 128], F32, name="ob")
                    nc.scalar.activation(
                        ob[0:w, :], tp[0:w, :],
                        mybir.ActivationFunctionType.Tanh)
                    nc.scalar.activation(
                        ob[0:w, :], ob[0:w, :],
                        mybir.ActivationFunctionType.Tanh)
                    nc.sync.dma_start(
                        ypad.rearrange("(t p) -> t p", p=M)[j * CH:j * CH + w],
                        ob[0:w, :])
            # extract valid pixels: drop the 2 pad cols per padded row
            rows_out = (n_tiles * M) // PW  # complete rows (224 when full)
            nc.sync.dma_start(
                y[b, 0, 0:rows_out, :],
                ypad[0:GTOT].rearrange("(h c) -> h c", c=PW)[0:rows_out, 0:W])


def _build(n_img=BPC, n_strips=None, enable_asserts=False, nrep=1):
    # num_devices=1: pure data-parallel SPMD, no collectives — each core runs
    # an independent single-device NEFF on its own input shard.
    nc = bacc.Bacc(
        "TRN2",
        target_bir_lowering=False,
        debug=False,
        enable_asserts=enable_asserts,
        num_devices=1,
    )
    DT_HBM = mybir.dt.float8e4 if DMA_CAST_FP8 else DT_X
    xp = nc.dram_tensor("xp", [n_img, 2, IC, PH, PW], DT_HBM,
                        kind="ExternalInput")
    wv = nc.dram_tensor("wv", [3, 128, 128], DT, kind="ExternalInput")
    ws = nc.dram_tensor("ws", [3, 128, 128], DT, kind="ExternalInput")
    bm = nc.dram_tensor("bias_mat", [128, 8, 128], F32,
                        kind="ExternalInput")
    y = nc.dram_tensor("y", [n_img, 1, H, W], F32, kind="ExternalOutput")
    with tile.TileContext(nc) as tc:
        _emit(nc, tc, y.ap(), xp.ap(), wv.ap(), ws.ap(), bm.ap(),
              n_img=n_img, n_strips=n_strips, nrep=nrep)
    nc.compile()
    return nc


def prep_inputs(x, weight, bias):
    """Host-side layout prep (numpy only)."""
    x = np.asarray(x, dtype=np.float32)
    weight = np.asarray(weight, dtype=np.float32)
    bias = np.asarray(bias, dtype=np.float32)
    nb = x.shape[0]
    xpad = np.zeros((nb, IC, PH, PW), dtype=np.float32)
    xpad[:, :, 1:225, 1:225] = x
    xpad = xpad.astype(
        ml_dtypes.float8_e4m3 if DMA_CAST_FP8 else DT_X_NP)
    # xp[:, 0] = xpad, xp[:, 1] = xpad shifted up one row (the strip tile's
    # lower half), so one 128-partition DMA covers both halves of a strip.
    xp = np.zeros((nb, 2, IC, PH, PW), dtype=xpad.dtype)
    xp[:, 0] = xpad
    xp[:, 1, :, 0:PH - 1] = xpad[:, :, 1:PH]
    xp = np.ascontiguousarray(xp)
    wv = np.zeros((3, 128, 128), dtype=np.float32)
    ws = np.zeros((3, 128, 128), dtype=np.float32)
    for kw in range(3):
        wv[kw, 0:64] = weight[:, :, 0, kw].T
        wv[kw, 64:128] = weight[:, :, 1, kw].T
        ws[kw, 0:64] = weight[:, :, 2, kw].T
        ws[kw, 64:128] = weight[:, :, 2, kw].T
    wv = np.ascontiguousarray(wv.astype(DT_NP))
    ws = np.ascontiguousarray(ws.astype(DT_NP))
    bm = np.ascontiguousarray(
        np.broadcast_to(bias[None, None, :], (128, 8, 128)).astype(np.float32))
    return xp, wv, ws, bm


def kernel(x, weight, bias):
    global LAST_RESULT
    xp, wv, ws, bm = prep_inputs(x, weight, bias)
    if "nc" not in _CACHE:
        _CACHE["nc"] = _build()
    nc = _CACHE["nc"]
    in_maps = []
    for c in range(N_CORES):
        m = {
            "xp": np.ascontiguousarray(xp[c * BPC:(c + 1) * BPC]),
            "wv": wv,
            "ws": ws,
            "bias_mat": bm,
        }
        in_maps.append(m)
    res = run_bass_kernel_spmd(nc, in_maps, core_ids=list(range(N_CORES)))
    LAST_RESULT = res
    y = np.concatenate([r["y"] for r in res.results], axis=0)
    return y



# revision 27
# speedup vs baseline: 1.4855x; 1.4597x over previous
"""Trainium2 Bass kernel: 3x3 SAME conv (64->128ch) + bias, double-tanh, min over
channels, for x[16,64,224,224] -> y[16,1,224,224].

Strategy
--------
- Data-parallel over batch: 16 images / 8 NeuronCores = 2 images per core.
  Same NEFF on every core, different input shard (no collectives).
- min_c tanh(tanh(v_c)) == tanh(tanh(min_c v_c)) (tanh is monotone), so the
  double tanh is applied only to the per-pixel channel-minimum.
- Conv as implicit GEMM with the *image patch stationary*: tiles of M=128
  consecutive pixels of the padded row-stream (row stride 226; the 2 pad cols
  per row produce garbage outputs that are dropped at extraction), accumulated
  into PSUM[128, 128oc] with matmuls lhsT=[K, 128 px], rhs=[K, 128 oc].
  M=128 keeps NumWeights==128 so the compiler enables Fast Weight Load.
  Output channels land on the PSUM free dim, so the channel-min is a native
  free-dim DVE reduction.
- A single strip tile per 28 output rows carries all nine taps: partitions
  0:64 = padded rows r..r+30, partitions 64:128 = rows r+1..r+31 (the same
  channels one row later).  Per pixel tile:
    * (kh=0,kw)|(kh=1,kw) K=128 pairs at window kw, kw = 0,1,2  (3 matmuls)
    * the three (kh=2,kw) taps are K=64 singles: for tile q from the upper
      half at window 2*226+kw, for tile q+4 from the lower half at window
      226+kw.  They are emitted alternating upper/lower so consecutive
      matmuls occupy disjoint PE row groups (tile_position row packing) and
      run concurrently into different PSUM banks -> ~1.5 matmuls/tile.
  Total PE stream ~4.5 x 128 columns per 128-pixel tile = the 9-tap minimum.
- Bias: each 8-tile / 2-bank PSUM group is pre-seeded with the bias, so conv
  matmuls accumulate onto it (start=False) and the DVE only does reduce_min
  over the oc axis.  The seed is a ScalarE copy into PSUM — legal because the
  PSUM has_written bits stay set from the slot's previous accumulation group;
  the first CPSUM_BUFS groups are instead seeded by K=1 rank-1 matmuls
  (ones stationary, bias streaming, start=True) to initialize the bits.
- Minima collect in a stage tile that is PE-transposed so pixels become the
  free dim, double-tanh'd on ScalarE, and DMA'd to a DRAM scratch in
  padded-stream order; one strided DRAM->DRAM DMA per image extracts the
  valid 224x224.
- Host-side prep (cheap numpy): zero-pad x to [.,64,228,226] bf16,
  pre-transpose the weights into [128,128] rhs tiles, tile the bias to
  [128,8,128] f32.
"""

import numpy as np
import ml_dtypes

import concourse.bass as bass
import concourse.mybir as mybir
import concourse.tile as tile
from concourse import bacc
from concourse.bass_utils import run_bass_kernel_spmd
from concourse.masks import make_identity

N_CORES = 8
B = 16
BPC = B // N_CORES  # images per core
IC, OC = 64, 128
H = W = 224
PW = 226    # padded row width in the pixel stream (= xp/xs width)
PH = 228    # xp rows (row r = image row r-1; rows 0, 225, 226, 227 zero)
R = 56      # output rows per strip
M = 128     # pixels per matmul tile
NPOS = R + 3   # padded-row positions in the main strip (upper half)
GTOT = H * PW  # padded-stream length per image (50624)
NT = -(-GTOT // M)  # tiles per image (396)
CH = 128    # stage chunk size (tiles per transpose)
F32 = mybir.dt.float32

DT = mybir.dt.bfloat16
DT_NP = ml_dtypes.bfloat16
# Image-data dtype.  fp8-e4m3 was tried (numerically fine — the conv output
# feeds tanh(tanh(min)) which saturates for randn inputs) but measured ~20%
# SLOWER than bf16: the 1-byte-sliding stationary windows appear to break
# Fast Weight Load, leaving the PE LDWEIGHTS-bound.  Keep bf16.
DT_X = mybir.dt.bfloat16
DT_X_NP = ml_dtypes.bfloat16

# Engine-attribution probes (invalid outputs; timing only).
PROBE_DVE_LIGHT = False  # reduce only 8 oc per psum slot (DVE ~5x lighter)
PROBE_DMA_LIGHT = False  # load strips only for s==0, reuse for all strips

# Store x strips as fp8 in HBM and cast to bf16 during the DMA (SWDGE cast
# path).  Halves HBM-side strip traffic; PE still sees bf16 operands.
DMA_CAST_FP8 = False
SINGLE_DMA_STRIP = True

STRIP_BUFS = 4
LOOKAHEAD = 2  # strips prefetched ahead of use (< STRIP_BUFS - 1)
# Seed the bias into each PSUM group with a ScalarE (ACT) copy instead of
# rank-1 matmuls.  PE accumulate (start=False) adds on top because the PSUM
# has_written bits persist from the slot's previous accumulation group; the
# first 3 groups (one per cpsum pool slot) still use the rank-1 matmul with
# start=True to put the bits into a known-set state.
ACT_BIAS = True
CPSUM_BUFS = 3  # PSUM group pool depth (ACT_BIAS seeds the first CPSUM_BUFS
                # groups via rank-1 matmuls to initialize each slot's bits)

_CACHE: dict = {}
LAST_RESULT = None  # BassKernelResults of the most recent run (for profiling)


def _strip_of(t):
    """Strip index owning tile t (by its first pixel row)."""
    return min((t * M) // PW // R, H // R - 1)


def _emit(nc: bass.Bass, tc: tile.TileContext, y, xp, wv, ws, bm,
          n_img=BPC, n_strips=None, nrep=1):
    """Emit the per-core program.

    y  : [n_img, 1, 224, 224] f32   ExternalOutput
    xp : [n_img, 2, 64, 228, 226] DT  padded input; [:,1] = shifted 1 row up
    wv : [3, 128, 128] DT   rhs tiles for the (kh=0 | kh=1) K-pairs, kw=0..2
    ws : [3, 128, 128] DT   ws[kw]: w(2,kw).T replicated in rows 0:64 and
                            64:128 (upper- and lower-half kh=2 singles)
    bm : [128, 8, 128] f32  bias broadcast to partitions and 8 bank slots
    """
    if n_strips is None:
        n_strips = H // R
    n_tiles = NT if n_strips == H // R else ((n_strips * R * PW) // M)
    with (
        tc.tile_pool(name="consts", bufs=1) as cpool,
        tc.tile_pool(name="strips", bufs=STRIP_BUFS) as spool,
        tc.tile_pool(name="stage", bufs=4) as stpool,
        tc.tile_pool(name="obuf", bufs=4) as opool,
        tc.tile_pool(name="dscratch", bufs=2, space="DRAM") as dpool,
        tc.tile_pool(name="cpsum", bufs=CPSUM_BUFS, space="PSUM") as cpsum,
        tc.tile_pool(name="tpsum", bufs=2, space="PSUM") as tpsum,
    ):
        # ---- constants ----
        identity = cpool.tile([128, 128], F32)
        make_identity(nc, identity)
        wv_sb = cpool.tile([128, 3, 128], DT)
        nc.sync.dma_start(wv_sb[:], wv.rearrange("t k n -> k t n"))
        ws_sb = cpool.tile([128, 3, 128], DT)
        nc.sync.dma_start(ws_sb[:], ws.rearrange("t k n -> k t n"))
        bias_mat = cpool.tile([128, 8, 128], F32)
        nc.sync.dma_start(bias_mat[:], bm)
        ones1 = cpool.tile([1, 128], DT)
        nc.gpsimd.memset(ones1[:], 1.0)
        bias_rhs = cpool.tile([1, 4, 128], DT)
        nc.vector.tensor_copy(bias_rhs[:], bias_mat[0:1, 0:4, :])

        grp_idx = 0
        img_list = [bb for _ in range(nrep) for bb in range(n_img)]
        dma = (nc.gpsimd.dma_start if DMA_CAST_FP8 else nc.sync.dma_start)

        # Strip prefetch: one tile per strip carries all nine taps (upper =
        # padded rows h0..h0+30, lower = rows h0+1..h0+31).  Strips are
        # loaded LOOKAHEAD ahead of use, across image boundaries, so the PE
        # never waits on a strip DMA.
        strip_seq = [(bb, sL) for bb in img_list for sL in range(n_strips)]
        strip_aps = [None] * len(strip_seq)
        next_load = [0]

        def load_ahead(upto):
            while next_load[0] <= min(upto, len(strip_seq) - 1):
                i = next_load[0]
                bb, sL = strip_seq[i]
                if PROBE_DMA_LIGHT and sL > 0:
                    strip_aps[i] = strip_aps[i - sL]
                else:
                    h0 = sL * R
                    ss = spool.tile([128, NPOS, PW], DT_X, name="ss")
                    if SINGLE_DMA_STRIP:
                        # ONE 128-partition dma_start per strip (reaches all
                        # 16 SBUF AXI ports; two 64-partition transfers would
                        # serialize on the FIFO rings).  xp carries the
                        # row-shifted copy at h=1 so both halves are one
                        # source.
                        dma(ss[:],
                            xp[bb].rearrange("h c r w -> (h c) r w")
                            [:, h0:h0 + NPOS, 0:PW])
                    else:
                        dma(ss[0:64], xp[bb, 0, :, h0:h0 + NPOS, 0:PW])
                        dma(ss[64:128],
                            xp[bb, 0, :, h0 + 1:h0 + NPOS + 1, 0:PW])
                    strip_aps[i] = ss.rearrange("p a c -> p (a c)")
                next_load[0] += 1

        for bi, b in enumerate(img_list):
            ypad = dpool.tile([NT * M], F32, name="ypad")
            stage_t = None
            psum_t = None
            ssf = None
            cur_strip = -1
            for t in range(n_tiles):
                s = _strip_of(t)
                if s != cur_strip:
                    cur_strip = s
                    si = bi * n_strips + s
                    load_ahead(si + LOOKAHEAD)
                    ssf = strip_aps[si]

                g = t * M - (s * R) * PW  # strip-local stream offset
                q = t % 8
                if q == 0:
                    psum_t = cpsum.tile([M, 8, 128], F32, name="psum_t")
                    if True:
                        pt_flat = psum_t.rearrange("p q n -> p (q n)")
                        if ACT_BIAS and grp_idx >= CPSUM_BUFS:
                            # ScalarE re-seed: values overwritten, has_written
                            # bits stay set from this slot's previous group,
                            # so start=False matmuls accumulate on top.
                            nc.scalar.activation(
                                pt_flat[:],
                                bias_mat.rearrange("p q n -> p (q n)")[:],
                                mybir.ActivationFunctionType.Copy)
                        else:
                            # seed both banks with bias via rank-1 matmuls
                            ngrp = min(8, n_tiles - t)
                            for bk in range(0, ngrp, 4):
                                w4 = min(4, ngrp - bk) * 128
                                nc.tensor.matmul(
                                    pt_flat[:, bk * 128: bk * 128 + w4],
                                    ones1[:],
                                    bias_rhs.rearrange("p q n -> p (q n)")
                                    [:, 0:w4],
                                    start=True, stop=False)
                    grp_idx += 1
                if q == 0:
                    pend = []  # deferred kh=2 singles: (q, g, ssf)
                for kw in range(3):      # (kh=0 | kh=1) pairs
                    nc.tensor.matmul(
                        psum_t[:, q], ssf[:, g + kw: g + kw + M], wv_sb[:, kw],
                        start=False, stop=False)
                # all three (kh=2, kw) taps ride in deferred K=64 singles:
                # tile q from the upper strip half (window 2*PW+kw), tile q+4
                # from the lower half (window PW+kw); emitted alternating so
                # disjoint PE row groups overlap, into different PSUM banks.
                pend.append((q, g, ssf))
                if q == 7 or t == n_tiles - 1:
                    ngrp = q + 1
                    for i in range(4):
                        for kw in range(3):
                            if i < ngrp:
                                qa, ga, fa = pend[i]
                                nc.tensor.matmul(
                                    psum_t[:, qa],
                                    fa[0:64, ga + 2 * PW + kw:
                                       ga + 2 * PW + kw + M],
                                    ws_sb[0:64, kw], start=False,
                                    stop=(kw == 2 and
                                          i == min(3, ngrp - 1)))
                            if i + 4 < ngrp:
                                qb, gb, fb = pend[i + 4]
                                nc.tensor.matmul(
                                    psum_t[:, qb],
                                    fb[64:128, gb + PW + kw:
                                       gb + PW + kw + M],
                                    ws_sb[64:128, kw], start=False,
                                    stop=(kw == 2 and i + 4 == ngrp - 1))

                if t % CH == 0:
                    stage_t = stpool.tile([128, CH], F32, name="stage_t")
                if q == 7 or t == n_tiles - 1:
                    nq = q + 1
                    cc = (t - q) % CH
                    # probe: span every psum slot (keeps the dependency on all
                    # of the group's matmuls — reading a partial slot set
                    # races DVE-R against PE-W in the same bank, which is a
                    # fatal PSUM collision) but read only 8 oc per slot.
                    pin = psum_t[:, 0:nq, 0:8] if PROBE_DVE_LIGHT \
                        else psum_t[:, 0:nq]
                    nc.vector.tensor_reduce(
                        out=stage_t[:, cc:cc + nq],
                        in_=pin,
                        axis=mybir.AxisListType.X,
                        op=mybir.AluOpType.min)
                if t % CH == CH - 1 or t == n_tiles - 1:
                    # chunk done: transpose -> tanh -> tanh -> scratch DMA
                    j = t // CH
                    w = t % CH + 1  # columns written in this chunk
                    tp = tpsum.tile([CH, 128], F32, name="tp")
                    nc.tensor.transpose(tp[0:w, :], stage_t[:, 0:w], identity)
                    ob = opool.tile([CH, 128], F32, name="ob")
                    nc.scalar.activation(
                        ob[0:w, :], tp[0:w, :],
                        mybir.ActivationFunctionType.Tanh)
                    nc.scalar.activation(
                        ob[0:w, :], ob[0:w, :],
                        mybir.ActivationFunctionType.Tanh)
                    nc.sync.dma_start(
                        ypad.rearrange("(t p) -> t p", p=M)[j * CH:j * CH + w],
                        ob[0:w, :])
            # extract valid pixels: drop the 2 pad cols per padded row
            rows_out = (n_tiles * M) // PW  # complete rows (224 when full)
            nc.sync.dma_start(
                y[b, 0, 0:rows_out, :],
                ypad[0:GTOT].rearrange("(h c) -> h c", c=PW)[0:rows_out, 0:W])


def _build(n_img=BPC, n_strips=None, enable_asserts=False, nrep=1):
    # num_devices=1: pure data-parallel SPMD, no collectives — each core runs
    # an independent single-device NEFF on its own input shard.
    nc = bacc.Bacc(
        "TRN2",
        target_bir_lowering=False,
        debug=False,
        enable_asserts=enable_asserts,
        num_devices=1,
    )
    DT_HBM = mybir.dt.float8e4 if DMA_CAST_FP8 else DT_X
    xp = nc.dram_tensor("xp", [n_img, 2, IC, PH, PW], DT_HBM,
                        kind="ExternalInput")
    wv = nc.dram_tensor("wv", [3, 128, 128], DT, kind="ExternalInput")
    ws = nc.dram_tensor("ws", [3, 128, 128], DT, kind="ExternalInput")
    bm = nc.dram_tensor("bias_mat", [128, 8, 128], F32,
                        kind="ExternalInput")
    y = nc.dram_tensor("y", [n_img, 1, H, W], F32, kind="ExternalOutput")
    with tile.TileContext(nc) as tc:
        _emit(nc, tc, y.ap(), xp.ap(), wv.ap(), ws.ap(), bm.ap(),
              n_img=n_img, n_strips=n_strips, nrep=nrep)
    nc.compile()
    return nc


def prep_inputs(x, weight, bias):
    """Host-side layout prep (numpy only)."""
    x = np.asarray(x, dtype=np.float32)
    weight = np.asarray(weight, dtype=np.float32)
    bias = np.asarray(bias, dtype=np.float32)
    nb = x.shape[0]
    xpad = np.zeros((nb, IC, PH, PW), dtype=np.float32)
    xpad[:, :, 1:225, 1:225] = x
    xpad = xpad.astype(
        ml_dtypes.float8_e4m3 if DMA_CAST_FP8 else DT_X_NP)
    # xp[:, 0] = xpad, xp[:, 1] = xpad shifted up one row (the strip tile's
    # lower half), so one 128-partition DMA covers both halves of a strip.
    xp = np.zeros((nb, 2, IC, PH, PW), dtype=xpad.dtype)
    xp[:, 0] = xpad
    xp[:, 1, :, 0:PH - 1] = xpad[:, :, 1:PH]
    xp = np.ascontiguousarray(xp)
    wv = np.zeros((3, 128, 128), dtype=np.float32)
    ws = np.zeros((3, 128, 128), dtype=np.float32)
    for kw in range(3):
        wv[kw, 0:64] = weight[:, :, 0, kw].T
        wv[kw, 64:128] = weight[:, :, 1, kw].T
        ws[kw, 0:64] = weight[:, :, 2, kw].T
        ws[kw, 64:128] = weight[:, :, 2, kw].T
    wv = np.ascontiguousarray(wv.astype(DT_NP))
    ws = np.ascontiguousarray(ws.astype(DT_NP))
    bm = np.ascontiguousarray(
        np.broadcast_to(bias[None, None, :], (128, 8, 128)).astype(np.float32))
    return xp, wv, ws, bm


def kernel(x, weight, bias):
    global LAST_RESULT
    xp, wv, ws, bm = prep_inputs(x, weight, bias)
    if "nc" not in _CACHE:
        _CACHE["nc"] = _build()
    nc = _CACHE["nc"]
    in_maps = []
    for c in range(N_CORES):
        m = {
            "xp": np.ascontiguousarray(xp[c * BPC:(c + 1) * BPC]),
            "wv": wv,
            "ws": ws,
            "bias_mat": bm,
        }
        in_maps.append(m)
    res = run_bass_kernel_spmd(nc, in_maps, core_ids=list(range(N_CORES)))
    LAST_RESULT = res
    y = np.concatenate([r["y"] for r in res.results], axis=0)
    return y

